# revision 7
# baseline (speedup 1.0000x reference)
"""Trainium2 Bass kernel for the sparse-attention nn.Module.

Sharding: 8 cores = 4 batches x 2 head-triples. Core i handles batch i//2 and
heads [3*(i%2), 3*(i%2)+3). Each core computes qkv 1x1 conv + depthwise 3x3 for
its 288 channels, the q-branch (W-row l2norm, superpixel projection multiply),
the per-head 32x32 attention Grams, the top-k soften / softmax mixture, and a
partial final 1x1 conv contribution out_w[:, its-96-v-channels] @ v. The host
sums the two partials per batch.

Per-core channel processing order (matmul M order):
  tile0 = [q(96) | v(0:32)]  tile1 = [k(96) | v(32:64)]  tile2 = [v(64:96)]
The 32-channel tail (tile2) is packed 4-chunks-per-128-partitions for the
depthwise conv, placed via matmul tile_position column groups.
"""

import numpy as np
import ml_dtypes
from contextlib import ExitStack

import concourse.bass as bass
import concourse.tile as tile
from concourse import bacc, mybir
from concourse.bass import ts
from concourse.bass_utils import run_bass_kernel_spmd

BF16 = mybir.dt.bfloat16
F32 = mybir.dt.float32
AX = mybir.AxisListType
OP = mybir.AluOpType
AF = mybir.ActivationFunctionType

DIM, HEADS, B, HIMG, WIMG = 192, 6, 4, 256, 256
C = 32
SMALL = 1e-6
N_CORES = 8
KKS = (C // 2, (C * 2) // 3, (C * 3) // 4, (C * 4) // 5)  # 16, 21, 24, 25

_NC_CACHE = {}


def build_nc(Himg=HIMG, hc=16):
    """Build the single-core SPMD program. Same program for all 8 cores."""
    key = (Himg, hc)
    if key in _NC_CACHE:
        return _NC_CACHE[key]

    W = WIMG
    NCH = Himg // hc            # number of row chunks
    assert NCH % 4 == 0
    ROWS = hc + 2               # chunk rows incl halo
    WP = W + 2                  # padded row length
    CW = hc * W                 # interior elems per chunk (per channel)
    NJ = (ROWS * W) // 512      # 512-wide matmul chunks per row-chunk
    NT = CW // 128              # 128-wide gram n-tiles per chunk
    HW = Himg * W

    nc = bacc.Bacc("TRN2", target_bir_lowering=False, debug=False,
                   num_devices=N_CORES)

    # ---- DRAM I/O ----
    x_d = nc.dram_tensor("x", [DIM, HW], BF16, kind="ExternalInput").ap()
    wq0_d = nc.dram_tensor("wq0", [128, 288], BF16, kind="ExternalInput").ap()
    wq1_d = nc.dram_tensor("wq1", [64, 288], BF16, kind="ExternalInput").ap()
    dww_d = nc.dram_tensor("dww", [128, 27], F32, kind="ExternalInput").ap()
    projw_d = nc.dram_tensor("projw", [27, 96], BF16, kind="ExternalInput").ap()
    oww_d = nc.dram_tensor("oww", [96, 192], BF16, kind="ExternalInput").ap()
    sp_d = nc.dram_tensor("sp", [3, Himg, W], F32, kind="ExternalInput").ap()
    par_d = nc.dram_tensor("par", [128, 8], F32, kind="ExternalInput").ap()
    id_d = nc.dram_tensor("ident", [96, 96], BF16, kind="ExternalInput").ap()
    out_d = nc.dram_tensor("out", [DIM, HW], F32, kind="ExternalOutput").ap()
    v_d = nc.dram_tensor("vscratch", [96, HW], BF16).ap()
    sp_pad = nc.dram_tensor("sppad", [3, Himg + 2, WP], BF16).ap()

    taps = [(dy, dx) for dy in (-1, 0, 1) for dx in (-1, 0, 1)]

    with tile.TileContext(nc) as tc, ExitStack() as top:
        pers = top.enter_context(tc.tile_pool(name="pers", bufs=1))
        gram_pool = top.enter_context(
            tc.tile_pool(name="gram_ps", bufs=1, space="PSUM"))

        # persistent tiles
        wq0 = pers.tile([128, 288], BF16, tag="wq0")
        wq1 = pers.tile([64, 288], BF16, tag="wq1")
        dww = pers.tile([128, 27], F32, tag="dww")
        projw = pers.tile([27, 96], BF16, tag="projw")
        oww = pers.tile([96, 192], BF16, tag="oww")
        par = pers.tile([128, 8], F32, tag="par")
        ident = pers.tile([96, 96], BF16, tag="ident")
        tail_u = pers.tile([128, ROWS, WP], BF16, tag="tail_u")
        qn_cols = pers.tile([96, NCH], F32, tag="qn_cols")
        kn_cols = pers.tile([96, NCH], F32, tag="kn_cols")
        sqscr = pers.tile([96, 256], F32, tag="sqscr")
        nscr = pers.tile([96, CW], BF16, tag="nscr")
        gram_ps = gram_pool.tile([96, 96], F32)

        for dst, src in ((wq0, wq0_d), (wq1, wq1_d), (dww, dww_d),
                         (projw, projw_d), (oww, oww_d), (par, par_d),
                         (ident, id_d)):
            nc.sync.dma_start(dst[:], src[:])

        # ---- P0: superpixel W-row l2norm into padded bf16 DRAM buffer ----
        with tc.tile_pool(name="spz", bufs=1) as spz_pool:
            zt = spz_pool.tile([1, WP], BF16, tag="zt")
            nc.vector.memset(zt[:], 0.0)
            for cc in range(3):
                nc.sync.dma_start(sp_pad[cc:cc + 1, 0:1, :], zt[:])
                nc.sync.dma_start(sp_pad[cc:cc + 1, Himg + 1:Himg + 2, :],
                                  zt[:])
        with tc.tile_pool(name="sp_pool", bufs=2) as sp_pool:
            rows_per_tile = 128
            total_rows = 3 * Himg
            r = 0
            while r < total_rows:
                nrows = min(rows_per_tile, total_rows - r)
                t = sp_pool.tile([128, W], F32, tag="sp_in")
                c0, h0 = r // Himg, r % Himg
                # rows r..r+nrows of the (c, h) flattening; may straddle c
                # boundary only if Himg not multiple of 128; handle per-c runs
                done = 0
                while done < nrows:
                    cc = (r + done) // Himg
                    hh = (r + done) % Himg
                    run = min(nrows - done, Himg - hh)
                    nc.sync.dma_start(t[done:done + run, :],
                                      sp_d[cc, hh:hh + run, :])
                    done += run
                sq = sp_pool.tile([128, W], F32, tag="sp_sq")
                nc.vector.tensor_tensor(sq[:nrows], t[:nrows], t[:nrows], OP.mult)
                ssum = sp_pool.tile([128, 1], F32, tag="sp_sum")
                nc.vector.tensor_reduce(ssum[:nrows], sq[:nrows], AX.X, OP.add)
                nrm = sp_pool.tile([128, 1], F32, tag="sp_nrm")
                nc.scalar.sqrt(nrm[:nrows], ssum[:nrows])
                rinv = sp_pool.tile([128, 1], F32, tag="sp_rinv")
                nc.vector.reciprocal(rinv[:nrows], nrm[:nrows])
                sc = sp_pool.tile([128, WP], BF16, tag="sp_sc")
                nc.vector.memset(sc[:, 0:1], 0.0)
                nc.vector.memset(sc[:, W + 1:W + 2], 0.0)
                nc.vector.tensor_scalar(sc[:nrows, 1:1 + W], t[:nrows],
                                        rinv[:nrows], None, OP.mult)
                done = 0
                while done < nrows:
                    cc = (r + done) // Himg
                    hh = (r + done) % Himg
                    run = min(nrows - done, Himg - hh)
                    nc.sync.dma_start(
                        sp_pad[cc:cc + 1, 1 + hh:1 + hh + run, :],
                        sc[done:done + run, :])
                    done += run
                r += nrows

        # ---- P1: chunk loop ----
        with ExitStack() as loop:
            x_pool = loop.enter_context(tc.tile_pool(name="x_pool", bufs=2))
            u_pool = loop.enter_context(tc.tile_pool(name="u_pool", bufs=2))
            dw_pool = loop.enter_context(tc.tile_pool(name="dw_pool", bufs=2))
            sm_pool = loop.enter_context(tc.tile_pool(name="sm_pool", bufs=2))
            im_pool = loop.enter_context(tc.tile_pool(name="im_pool", bufs=2))
            rq_pool = loop.enter_context(tc.tile_pool(name="rq_pool", bufs=2))
            tqk_pool = loop.enter_context(tc.tile_pool(name="tqk_pool", bufs=3))
            to_pool = loop.enter_context(tc.tile_pool(name="to_pool", bufs=2))
            qkv_ps = loop.enter_context(
                tc.tile_pool(name="qkv_ps", bufs=3, space="PSUM"))
            sp_ps = loop.enter_context(
                tc.tile_pool(name="sp_ps", bufs=1, space="PSUM"))
            pst_ps = loop.enter_context(
                tc.tile_pool(name="pst_ps", bufs=2, space="PSUM"))

            drains = [nc.scalar, nc.vector]

            for ci in range(NCH):
                r0 = ci * hc
                g = ci % 4

                # -- x chunk load (rows r0-1 .. r0+hc, clamped) --
                xs0 = x_pool.tile([128, ROWS * W], BF16, tag="xs0")
                xs1 = x_pool.tile([64, ROWS * W], BF16, tag="xs1")
                lo = (r0 - 1) * W
                hi = (r0 + hc + 1) * W
                dst0 = 0
                if lo < 0:
                    nc.vector.memset(xs0[:, 0:W], 0.0)
                    nc.vector.memset(xs1[:, 0:W], 0.0)
                    dst0, lo = W, 0
                if hi > HW:
                    nc.vector.memset(xs0[:, ROWS * W - W:], 0.0)
                    nc.vector.memset(xs1[:, ROWS * W - W:], 0.0)
                    hi = HW
                nc.sync.dma_start(xs0[:, dst0:dst0 + (hi - lo)], x_d[0:128, lo:hi])
                nc.sync.dma_start(xs1[:, dst0:dst0 + (hi - lo)], x_d[128:192, lo:hi])

                # -- qkv 1x1 conv --
                u0 = u_pool.tile([128, ROWS, WP], BF16, tag="u0")
                u1 = u_pool.tile([128, ROWS, WP], BF16, tag="u1")
                for u_t in (u0, u1):
                    nc.gpsimd.memset(u_t[:, :, 0:1], 0.0)
                    nc.gpsimd.memset(u_t[:, :, W + 1:W + 2], 0.0)
                if g == 0:
                    nc.gpsimd.memset(tail_u[:, :, 0:1], 0.0)
                    nc.gpsimd.memset(tail_u[:, :, W + 1:W + 2], 0.0)
                for j in range(NJ):
                    rr = (j * 512) // W  # first row of this 512 chunk
                    for mt in range(3):
                        ps = qkv_ps.tile([128, 2, 256], F32, tag="qkv_ps")
                        if mt < 2:
                            po = ps[:, :, :]
                            tp = None
                            msl = ts(mt, 128)
                        else:
                            po = ps[32 * g:32 * g + 32, :, :]
                            tp = (0, 32 * g)
                            msl = slice(256, 288)
                        nc.tensor.matmul(po, wq0[:, msl], xs0[:, ts(j, 512)],
                                         start=True, stop=False,
                                         tile_position=tp)
                        nc.tensor.matmul(po, wq1[:, msl], xs1[:, ts(j, 512)],
                                         start=False, stop=True,
                                         tile_position=tp)
                        if mt == 0:
                            dst = u0[:, rr:rr + 2, 1:1 + W]
                        elif mt == 1:
                            dst = u1[:, rr:rr + 2, 1:1 + W]
                        else:
                            dst = tail_u[32 * g:32 * g + 32, rr:rr + 2, 1:1 + W]
                        eng = drains[(j + mt) % 2]
                        if eng is nc.scalar:
                            nc.scalar.copy(dst, po)
                        else:
                            nc.vector.tensor_copy(dst, po)

                # -- depthwise 3x3 (tiles 0, 1 on DVE) --
                dwout0 = dw_pool.tile([128, hc, W], BF16, tag="dwout0")
                dwout1 = dw_pool.tile([128, hc, W], BF16, tag="dwout1")
                for tidx, (u_t, dwout) in enumerate(((u0, dwout0), (u1, dwout1))):
                    for t, (dy, dx) in enumerate(taps):
                        iv = u_t[:, 1 + dy:1 + hc + dy, 1 + dx:1 + W + dx]
                        w_ap = dww[:, 9 * tidx + t:9 * tidx + t + 1]
                        if t == 0:
                            nc.vector.tensor_scalar(dwout[:], iv, w_ap, None,
                                                    OP.mult)
                        else:
                            nc.vector.scalar_tensor_tensor(
                                dwout[:], iv, w_ap, dwout[:], OP.mult, OP.add)

                # -- depthwise for packed tail every 4th chunk --
                if g == 3:
                    tailout = to_pool.tile([128, hc, W], BF16, tag="tailout")
                    for t, (dy, dx) in enumerate(taps):
                        iv = tail_u[:, 1 + dy:1 + hc + dy, 1 + dx:1 + W + dx]
                        w_ap = dww[:, 18 + t:19 + t]
                        if t == 0:
                            nc.vector.tensor_scalar(tailout[:], iv, w_ap, None,
                                                    OP.mult)
                        else:
                            nc.vector.scalar_tensor_tensor(
                                tailout[:], iv, w_ap, tailout[:], OP.mult,
                                OP.add)
                    for gg in range(4):
                        cols = (ci - 3 + gg) * CW
                        nc.sync.dma_start(v_d[64:96, cols:cols + CW],
                                          tailout[32 * gg:32 * gg + 32, :, :])

                # -- v rows out --
                nc.sync.dma_start(v_d[0:32, ci * CW:(ci + 1) * CW],
                                  dwout0[96:128, :, :])
                nc.sync.dma_start(v_d[32:64, ci * CW:(ci + 1) * CW],
                                  dwout1[96:128, :, :])

                # -- q branch: W-row l2norm --
                rqc = rq_pool.tile([96, hc], F32, tag="rqc")
                for s in range(hc):
                    nc.scalar.activation(sqscr[:], dwout0[0:96, s, :], AF.Square,
                                         accum_out=rqc[:, s:s + 1])
                rnorm = rq_pool.tile([96, hc], F32, tag="rnorm")
                nc.scalar.sqrt(rnorm[:], rqc[:])
                rq = rq_pool.tile([96, hc], F32, tag="rq")
                nc.vector.reciprocal(rq[:], rnorm[:])

                # -- sp_mean for this chunk: im2col + matmul --
                imc = im_pool.tile([27, hc, W], BF16, tag="imc")
                for t, (dy, dx) in enumerate(taps):
                    nc.sync.dma_start(
                        imc[3 * t:3 * t + 3, :, :],
                        sp_pad[0:3, 1 + r0 + dy:1 + r0 + hc + dy,
                               1 + dx:1 + W + dx])
                spm = sm_pool.tile([96, hc, W], BF16, tag="spm")
                for j in range(CW // 512):
                    rr = (j * 512) // W
                    ps = sp_ps.tile([96, 2, 256], F32, tag="sp_ps")
                    nc.tensor.matmul(ps[:], projw[:], imc[:, rr:rr + 2, :],
                                     start=True, stop=True)
                    nc.scalar.copy(spm[:, rr:rr + 2, :], ps[:])

                # -- qs_un = (qdw * rq_seg) * sp_mean, in place over dwout0 --
                for s in range(hc):
                    nc.vector.scalar_tensor_tensor(
                        dwout0[0:96, s, :], dwout0[0:96, s, :], rq[:, s:s + 1],
                        spm[:, s, :], OP.mult, OP.mult)

                # -- norm accumulators for hw-l2norm (post-gram absorb) --
                nc.scalar.activation(nscr[:], dwout0[0:96, :, :], AF.Square,
                                     accum_out=qn_cols[:, ci:ci + 1])
                nc.scalar.activation(nscr[:], dwout1[0:96, :, :], AF.Square,
                                     accum_out=kn_cols[:, ci:ci + 1])

                # -- transposes + gram accumulation --
                for tt4 in range(NT // 4):
                    pst = pst_ps.tile([128, 8, 96], BF16, tag="pst")
                    for k in range(4):
                        t = tt4 * 4 + k
                        s, half = t // 2, (t % 2) * 128
                        nc.tensor.transpose(
                            pst[:, 2 * k, :],
                            dwout0[0:96, s, half:half + 128], ident[:])
                        nc.tensor.transpose(
                            pst[:, 2 * k + 1, :],
                            dwout1[0:96, s, half:half + 128], ident[:])
                    qkt = tqk_pool.tile([128, 8, 96], BF16, tag="qkt")
                    nc.scalar.copy(qkt[:], pst[:])
                    for k in range(4):
                        t = tt4 * 4 + k
                        first = (ci == 0 and t == 0)
                        last = (ci == NCH - 1 and t == NT - 1)
                        nc.tensor.matmul(gram_ps[:], qkt[:, 2 * k, :],
                                         qkt[:, 2 * k + 1, :],
                                         start=first, stop=last)

        # ---- P2: attention block ----
        p2 = top.enter_context(tc.tile_pool(name="p2", bufs=1))
        ft_ps_pool = top.enter_context(
            tc.tile_pool(name="ft_ps", bufs=1, space="PSUM"))

        qn = p2.tile([96, 1], F32, tag="qn")
        kn = p2.tile([96, 1], F32, tag="kn")
        nc.vector.tensor_reduce(qn[:], qn_cols[:], AX.X, OP.add)
        nc.vector.tensor_reduce(kn[:], kn_cols[:], AX.X, OP.add)
        nc.scalar.sqrt(qn[:], qn[:])
        nc.scalar.sqrt(kn[:], kn[:])
        rqf = p2.tile([96, 1], F32, tag="rqf")
        rkf = p2.tile([96, 1], F32, tag="rkf")
        nc.vector.reciprocal(rqf[:], qn[:])
        nc.vector.reciprocal(rkf[:], kn[:])
        # fold temperature into rq
        nc.vector.tensor_tensor(rqf[:], rqf[:], par[0:96, 5:6], OP.mult)

        gram_sb = p2.tile([96, 96], F32, tag="gram_sb")
        nc.vector.tensor_copy(gram_sb[:], gram_ps[:])

        # rk broadcast + 32-block transpose -> rkT[(h,c), d] = rk[h, d]
        rkb = p2.tile([96, 32], F32, tag="rkb")
        nc.vector.tensor_scalar(rkb[:], rkf[:].broadcast_to((96, 32)), 1.0,
                                None, OP.mult)
        rkT = p2.tile([96, 32], F32, tag="rkT")
        nc.vector.transpose(rkT[:], rkb[:])

        attn = p2.tile([96, 32], F32, tag="attn")
        for h in range(3):
            nc.vector.tensor_copy(attn[32 * h:32 * h + 32, :],
                                  gram_sb[32 * h:32 * h + 32,
                                          32 * h:32 * h + 32])
        nc.vector.tensor_scalar(attn[:], attn[:], rqf[:], None, OP.mult)
        nc.vector.tensor_tensor(attn[:], attn[:], rkT[:], OP.mult)

        # ranks
        R = p2.tile([96, 32], F32, tag="R")
        gescr = p2.tile([96, 32], F32, tag="gescr")
        for d in range(32):
            nc.vector.tensor_scalar(gescr[:], attn[:], attn[:, d:d + 1], None,
                                    OP.is_ge)
            nc.vector.tensor_reduce(R[:, d:d + 1], gescr[:], AX.X, OP.add)

        # A = c0 * relu(attn) + sum_i c_i * softmax(soften_i(attn))
        A = p2.tile([96, 32], F32, tag="A")
        nc.vector.tensor_scalar(A[:], attn[:], 0.0, par[0:96, 4:5], OP.max,
                                OP.mult)
        mscr = p2.tile([96, 32], F32, tag="mscr")
        escr = p2.tile([96, 32], F32, tag="escr")
        mx = p2.tile([96, 1], F32, tag="mx")
        se = p2.tile([96, 1], F32, tag="se")
        coef = p2.tile([96, 1], F32, tag="coef")
        for i, kk in enumerate(KKS):
            nc.vector.tensor_scalar(mscr[:], R[:], float(kk), None, OP.is_le)
            nc.vector.tensor_scalar(mscr[:], mscr[:], 1.0 - SMALL, SMALL,
                                    OP.mult, OP.add)
            nc.vector.tensor_tensor(mscr[:], attn[:], mscr[:], OP.mult)
            nc.vector.tensor_reduce(mx[:], mscr[:], AX.X, OP.max)
            nc.vector.tensor_scalar(mscr[:], mscr[:], mx[:], None, OP.subtract)
            nc.scalar.activation(escr[:], mscr[:], AF.Exp, accum_out=se[:])
            nc.vector.reciprocal(se[:], se[:])
            nc.vector.tensor_tensor(coef[:], se[:], par[0:96, i:i + 1], OP.mult)
            nc.vector.scalar_tensor_tensor(A[:], escr[:], coef[:], A[:],
                                           OP.mult, OP.add)

        # block-diag A (bf16) and F^T = A_bd^T-contraction with out_w slice
        abd = p2.tile([96, 96], BF16, tag="abd")
        nc.vector.memset(abd[:], 0.0)
        for h in range(3):
            nc.vector.tensor_copy(abd[32 * h:32 * h + 32, 32 * h:32 * h + 32],
                                  A[32 * h:32 * h + 32, :])
        ft_ps = ft_ps_pool.tile([96, 192], F32)
        nc.tensor.matmul(ft_ps[:], abd[:], oww[:], start=True, stop=True)
        ftsb = p2.tile([96, 192], BF16, tag="ftsb")
        nc.scalar.copy(ftsb[:], ft_ps[:])

        tc.strict_bb_all_engine_barrier()

        # ---- P3: partial = F @ v ----
        with ExitStack() as p3:
            v_pool = p3.enter_context(tc.tile_pool(name="v_pool", bufs=3))
            o_pool = p3.enter_context(tc.tile_pool(name="o_pool", bufs=2))
            o_ps = p3.enter_context(
                tc.tile_pool(name="o_ps", bufs=2, space="PSUM"))
            for j in range(HW // 512):
                vsb = v_pool.tile([96, 512], BF16, tag="vsb")
                nc.sync.dma_start(vsb[:], v_d[:, ts(j, 512)])
                ps0 = o_ps.tile([128, 512], F32, tag="ops0")
                ps1 = o_ps.tile([64, 512], F32, tag="ops1")
                nc.tensor.matmul(ps0[:], ftsb[:, 0:128], vsb[:], start=True,
                                 stop=True)
                nc.tensor.matmul(ps1[:], ftsb[:, 128:192], vsb[:], start=True,
                                 stop=True)
                ob0 = o_pool.tile([128, 512], F32, tag="ob0")
                ob1 = o_pool.tile([64, 512], F32, tag="ob1")
                if j % 2 == 0:
                    nc.scalar.copy(ob0[:], ps0[:])
                    nc.vector.tensor_copy(ob1[:], ps1[:])
                else:
                    nc.vector.tensor_copy(ob0[:], ps0[:])
                    nc.scalar.copy(ob1[:], ps1[:])
                nc.sync.dma_start(out_d[0:128, ts(j, 512)], ob0[:])
                nc.sync.dma_start(out_d[128:192, ts(j, 512)], ob1[:])

    nc.compile()
    _NC_CACHE[key] = nc
    return nc


def core_inputs(i, x, superpixel_features, qkv_w, dw_w, proj_w, out_w,
                temperature, attn_scales, w_mix, Himg=HIMG):
    """Host-side slicing/packing of the full inputs for core i."""
    bf = ml_dtypes.bfloat16
    b, grp = i // 2, i % 2
    hs = 96 * grp
    HW = Himg * WIMG

    q_idx = np.arange(hs, hs + 96)
    k_idx = np.arange(DIM + hs, DIM + hs + 96)
    v_idx = np.arange(2 * DIM + hs, 2 * DIM + hs + 96)
    sel = np.concatenate([q_idx, v_idx[:32], k_idx, v_idx[32:64], v_idx[64:]])

    wq = np.asarray(qkv_w)[:, :, 0, 0]          # [576, 192]
    wqT = wq[sel].T.astype(bf)                  # [192, 288]

    w9 = np.asarray(dw_w)[:, 0].reshape(3 * DIM, 9)[sel]   # [288, 9]
    dww = np.zeros((128, 27), np.float32)
    dww[:, 0:9] = w9[0:128]
    dww[:, 9:18] = w9[128:256]
    dww[:, 18:27] = np.tile(w9[256:288], (4, 1))

    pw = np.asarray(proj_w)[hs:hs + 96]          # [96, 3, 3, 3] (oc, ic, ky, kx)
    projw = np.transpose(pw, (2, 3, 1, 0)).reshape(27, 96).astype(bf)

    oww = np.asarray(out_w)[:, hs:hs + 96, 0, 0].T.astype(bf)   # [96, 192]

    wmx = np.exp(np.asarray(w_mix) - np.max(w_mix))
    wmx = (wmx / wmx.sum()).astype(np.float64)
    S = np.asarray(attn_scales, np.float64)
    par = np.zeros((128, 8), np.float32)
    for ii in range(4):
        par[:, ii] = wmx[1] * S[ii]
    par[:, 4] = wmx[0] * S.sum()
    temps = np.asarray(temperature).reshape(HEADS)[3 * grp:3 * grp + 3]
    par[0:96, 5] = np.repeat(temps, 32)

    return {
        "x": np.asarray(x)[b].reshape(DIM, HW).astype(bf),
        "wq0": wqT[:128].copy(),
        "wq1": wqT[128:].copy(),
        "dww": dww,
        "projw": projw,
        "oww": oww,
        "sp": np.asarray(superpixel_features)[0, :, :Himg, :].astype(np.float32).copy(),
        "par": par,
        "ident": np.eye(96, dtype=bf),
    }


def kernel(x, superpixel_features, qkv_w, dw_w, proj_w, out_w, temperature,
           attn_scales, w_mix):
    nc = build_nc(HIMG, 16)
    in_maps = [
        core_inputs(i, x, superpixel_features, qkv_w, dw_w, proj_w, out_w,
                    temperature, attn_scales, w_mix)
        for i in range(N_CORES)
    ]
    res = run_bass_kernel_spmd(nc, in_maps, list(range(N_CORES)))
    out = np.empty((B, DIM, HIMG, WIMG), np.float32)
    for b in range(B):
        part = res.results[2 * b]["out"] + res.results[2 * b + 1]["out"]
        out[b] = part.reshape(DIM, HIMG, WIMG)
    return out


# revision 11
# speedup vs baseline: 1.4273x; 1.4273x over previous
"""Trainium2 Bass kernel for the sparse-attention nn.Module.

Sharding: 8 cores = 4 batches x 2 head-triples. Core i handles batch i//2 and
heads [3*(i%2), 3*(i%2)+3). Each core computes the qkv 1x1 conv + depthwise
3x3 for its 288 channels, the q-branch (W-row l2norm, superpixel projection
multiply), per-head 32x32 attention Grams, the top-k soften / softmax mix, and
a partial final 1x1 conv contribution out_w[:, its-96-v-channels] @ v. The
host sums the two partials per batch.

Per-core channel order (qkv matmul M order):
  tile0 = [q(96) | v(0:32)]   -> depthwise on DVE (flat padded grid,
                                 tensor_scalar 4x + tensor_tensor 2x, u_odd
                                 shadow copy for odd-offset taps)
  tile1 = [k(96) | v(32:64)]  -> depthwise as 9 diagonal matmuls on PE
                                 accumulating in PSUM
  tile2 = [v(64:96)]          -> packed 4-chunks-per-128-partitions (via
                                 matmul tile_position col groups), depthwise
                                 on PE every 4th chunk
qs/kh Gram operands are transposed with tiled xbar DMA transposes
(out[p,t,c] = in[c, 128*t+p]) straight into SBUF.
"""

import numpy as np
import ml_dtypes
from contextlib import ExitStack

import concourse.bass as bass
import concourse.tile as tile
from concourse import bacc, mybir
from concourse.bass import ts
from concourse.bass_utils import run_bass_kernel_spmd

BF16 = mybir.dt.bfloat16
F32 = mybir.dt.float32
AX = mybir.AxisListType
OP = mybir.AluOpType
AF = mybir.ActivationFunctionType

DIM, HEADS, B, HIMG, WIMG = 192, 6, 4, 256, 256
C = 32
SMALL = 1e-6
N_CORES = 8
KKS = (C // 2, (C * 2) // 3, (C * 3) // 4, (C * 4) // 5)  # 16, 21, 24, 25

_NC_CACHE = {}


def build_nc(Himg=HIMG, hc=16):
    """Build the single-core SPMD program. Same program for all 8 cores."""
    key = (Himg, hc)
    if key in _NC_CACHE:
        return _NC_CACHE[key]

    W = WIMG
    NCH = Himg // hc            # number of row chunks
    assert NCH % 4 == 0
    ROWS = hc + 4               # margin + halo + hc + halo + margin
    WP = W + 4                  # 2 pad cols each side (even offsets)
    CW = hc * W                 # interior elems per chunk per channel
    NJ = ((hc + 2) * W) // 512  # 512-wide matmul chunks per row chunk
    NT = CW // 128              # 128-wide gram n-tiles per chunk
    LEN = hc * WP               # flat dw compute length
    HW = Himg * W
    taps = [(dy, dx) for dy in (-1, 0, 1) for dx in (-1, 0, 1)]

    nc = bacc.Bacc("TRN2", target_bir_lowering=False, debug=False,
                   num_devices=N_CORES)

    # ---- DRAM I/O ----
    x_d = nc.dram_tensor("x", [DIM, HW], BF16, kind="ExternalInput").ap()
    wq0_d = nc.dram_tensor("wq0", [128, 288], BF16, kind="ExternalInput").ap()
    wq1_d = nc.dram_tensor("wq1", [64, 288], BF16, kind="ExternalInput").ap()
    dww_d = nc.dram_tensor("dww", [128, 9], F32, kind="ExternalInput").ap()
    dwdiag_d = nc.dram_tensor("dwdiag", [128, 18, 128], BF16,
                              kind="ExternalInput").ap()
    projw_d = nc.dram_tensor("projw", [27, 96], BF16, kind="ExternalInput").ap()
    oww_d = nc.dram_tensor("oww", [96, 192], BF16, kind="ExternalInput").ap()
    sp_d = nc.dram_tensor("sp", [3, Himg, W], F32, kind="ExternalInput").ap()
    par_d = nc.dram_tensor("par", [128, 8], F32, kind="ExternalInput").ap()
    out_d = nc.dram_tensor("out", [DIM, HW], F32, kind="ExternalOutput").ap()
    v_d = nc.dram_tensor("vscratch", [96, HW], BF16).ap()
    sp_pad = nc.dram_tensor("sppad", [3, Himg + 2, WP], BF16).ap()

    with tile.TileContext(nc) as tc, ExitStack() as top:
        pers = top.enter_context(tc.tile_pool(name="pers", bufs=1))
        mid = top.enter_context(ExitStack())
        gram_pool = mid.enter_context(
            tc.tile_pool(name="gram_ps", bufs=1, space="PSUM"))

        wq0 = pers.tile([128, 288], BF16, tag="wq0")
        wq1 = pers.tile([64, 288], BF16, tag="wq1")
        dww = pers.tile([128, 9], F32, tag="dww")
        dwdiag = pers.tile([128, 18, 128], BF16, tag="dwdiag")
        projw = pers.tile([27, 96], BF16, tag="projw")
        oww = pers.tile([96, 192], BF16, tag="oww")
        par = pers.tile([128, 8], F32, tag="par")
        tail_u = pers.tile([128, ROWS, WP], BF16, tag="tail_u")
        qn_cols = pers.tile([96, NCH], F32, tag="qn_cols")
        kn_cols = pers.tile([96, NCH], F32, tag="kn_cols")
        gram_ps = gram_pool.tile([96, 96], F32)

        for dst, src in ((wq0, wq0_d), (wq1, wq1_d), (dww, dww_d),
                         (dwdiag, dwdiag_d), (projw, projw_d), (oww, oww_d),
                         (par, par_d)):
            nc.sync.dma_start(dst[:], src[:])

        # ---- P0: superpixel W-row l2norm into padded bf16 DRAM buffer ----
        with tc.tile_pool(name="spz", bufs=1) as spz_pool:
            zt = spz_pool.tile([1, WP], BF16, tag="zt")
            nc.vector.memset(zt[:], 0.0)
            for cc in range(3):
                nc.sync.dma_start(sp_pad[cc:cc + 1, 0:1, :], zt[:])
                nc.sync.dma_start(sp_pad[cc:cc + 1, Himg + 1:Himg + 2, :],
                                  zt[:])
        with tc.tile_pool(name="sp_pool", bufs=2) as sp_pool:
            total_rows = 3 * Himg
            r = 0
            while r < total_rows:
                nrows = min(128, total_rows - r)
                t = sp_pool.tile([128, W], F32, tag="sp_in")
                done = 0
                while done < nrows:
                    cc = (r + done) // Himg
                    hh = (r + done) % Himg
                    run = min(nrows - done, Himg - hh)
                    nc.sync.dma_start(t[done:done + run, :],
                                      sp_d[cc, hh:hh + run, :])
                    done += run
                sq = sp_pool.tile([128, W], F32, tag="sp_sq")
                nc.vector.tensor_tensor(sq[:nrows], t[:nrows], t[:nrows],
                                        OP.mult)
                ssum = sp_pool.tile([128, 1], F32, tag="sp_sum")
                nc.vector.tensor_reduce(ssum[:nrows], sq[:nrows], AX.X, OP.add)
                nrm = sp_pool.tile([128, 1], F32, tag="sp_nrm")
                nc.scalar.sqrt(nrm[:nrows], ssum[:nrows])
                rinv = sp_pool.tile([128, 1], F32, tag="sp_rinv")
                nc.vector.reciprocal(rinv[:nrows], nrm[:nrows])
                sc = sp_pool.tile([128, WP], BF16, tag="sp_sc")
                nc.vector.memset(sc[:, 0:2], 0.0)
                nc.vector.memset(sc[:, W + 2:W + 4], 0.0)
                nc.vector.tensor_scalar(sc[:nrows, 2:2 + W], t[:nrows],
                                        rinv[:nrows], None, OP.mult)
                done = 0
                while done < nrows:
                    cc = (r + done) // Himg
                    hh = (r + done) % Himg
                    run = min(nrows - done, Himg - hh)
                    nc.sync.dma_start(
                        sp_pad[cc:cc + 1, 1 + hh:1 + hh + run, :],
                        sc[done:done + run, :])
                    done += run
                r += nrows

        # ---- P1: chunk loop ----
        with ExitStack() as loop:
            x_pool = loop.enter_context(tc.tile_pool(name="x_pool", bufs=2))
            u_pool = loop.enter_context(tc.tile_pool(name="u_pool", bufs=2))
            uo_pool = loop.enter_context(tc.tile_pool(name="uo_pool", bufs=1))
            acc_pool = loop.enter_context(tc.tile_pool(name="acc_pool", bufs=1))
            dw_pool = loop.enter_context(tc.tile_pool(name="dw_pool", bufs=2))
            sm_pool = loop.enter_context(tc.tile_pool(name="sm_pool", bufs=1))
            im_pool = loop.enter_context(tc.tile_pool(name="im_pool", bufs=1))
            rq_pool = loop.enter_context(tc.tile_pool(name="rq_pool", bufs=2))
            tr_pool = loop.enter_context(tc.tile_pool(name="tr_pool", bufs=2))
            to_pool = loop.enter_context(tc.tile_pool(name="to_pool", bufs=1))
            qkv_ps = loop.enter_context(
                tc.tile_pool(name="qkv_ps", bufs=2, space="PSUM"))
            dg_ps = loop.enter_context(
                tc.tile_pool(name="dg_ps", bufs=2, space="PSUM"))

            for ci in range(NCH):
                r0 = ci * hc
                g = ci % 4

                # -- x chunk load (rows r0-1 .. r0+hc, clamped) --
                xs0 = x_pool.tile([128, (hc + 2) * W], BF16, tag="xs0")
                xs1 = x_pool.tile([64, (hc + 2) * W], BF16, tag="xs1")
                lo = (r0 - 1) * W
                hi = (r0 + hc + 1) * W
                dst0 = 0
                if lo < 0:
                    nc.gpsimd.memset(xs0[:, 0:W], 0.0)
                    nc.gpsimd.memset(xs1[:, 0:W], 0.0)
                    dst0, lo = W, 0
                if hi > HW:
                    nc.gpsimd.memset(xs0[:, (hc + 1) * W:], 0.0)
                    nc.gpsimd.memset(xs1[:, (hc + 1) * W:], 0.0)
                    hi = HW
                nc.sync.dma_start(xs0[:, dst0:dst0 + (hi - lo)],
                                  x_d[0:128, lo:hi])
                nc.sync.dma_start(xs1[:, dst0:dst0 + (hi - lo)],
                                  x_d[128:192, lo:hi])

                # -- qkv 1x1 conv (mt-outer for weight reuse) --
                u0 = u_pool.tile([128, ROWS, WP], BF16, tag="u0")
                u1 = u_pool.tile([128, ROWS, WP], BF16, tag="u1")
                for u_t in (u0, u1):
                    nc.gpsimd.memset(u_t[:, :, 0:2], 0.0)
                    nc.gpsimd.memset(u_t[:, :, W + 2:W + 4], 0.0)
                nc.gpsimd.memset(u0[:, 0:1, 2:2 + W], 0.0)
                nc.gpsimd.memset(u0[:, ROWS - 1:ROWS, 2:2 + W], 0.0)
                if g == 0:
                    nc.gpsimd.memset(tail_u[:, :, 0:2], 0.0)
                    nc.gpsimd.memset(tail_u[:, :, W + 2:W + 4], 0.0)
                for mt in range(3):
                    if mt < 2:
                        tp = None
                        msl = ts(mt, 128)
                        pbase, psz = 0, 128
                    else:
                        tp = (0, 32 * g)
                        msl = slice(256, 288)
                        pbase, psz = 32 * g, 32
                    jj = 0
                    while jj < NJ:
                        w2 = min(2, NJ - jj)  # n-chunks in this batch
                        ps = qkv_ps.tile([128, 4, 256], F32, tag="qkv_ps")
                        for u in range(w2):
                            po = ps[pbase:pbase + psz, 2 * u:2 * u + 2, :]
                            nc.tensor.matmul(po, wq0[:, msl],
                                             xs0[:, ts(jj + u, 512)],
                                             start=True, stop=False,
                                             tile_position=tp)
                            nc.tensor.matmul(po, wq1[:, msl],
                                             xs1[:, ts(jj + u, 512)],
                                             start=False, stop=True,
                                             tile_position=tp)
                        rr = 1 + jj * 2  # dest row in u (margin offset 1)
                        nr = 2 * w2
                        if mt == 0:
                            dst = u0[:, rr:rr + nr, 2:2 + W]
                        elif mt == 1:
                            dst = u1[:, rr:rr + nr, 2:2 + W]
                        else:
                            dst = tail_u[pbase:pbase + 32, rr:rr + nr, 2:2 + W]
                        src = ps[pbase:pbase + psz, 0:nr, :]
                        if (jj // 2 + mt) % 2 == 0:
                            nc.scalar.copy(dst, src)
                        else:
                            nc.vector.tensor_copy(dst, src)
                        jj += w2

                # -- depthwise tile0 on DVE: flat padded grid --
                u0f = u0[:].rearrange("p a b -> p (a b)")
                nodd = 2 * WP + WP + 1 + LEN  # covers max odd-tap view
                u_odd = uo_pool.tile([128, nodd], BF16, tag="u_odd")
                nc.vector.tensor_copy(u_odd[:], u0f[:, 1:1 + nodd])
                acc = acc_pool.tile([128, LEN], BF16, tag="acc")
                tmp = acc_pool.tile([128, LEN], BF16, tag="tmp")
                base = 2 * WP  # out row 2 (first interior), col 0
                for t, (dy, dx) in enumerate(taps):
                    off = base + dy * WP + dx
                    w_ap = dww[:, t:t + 1]
                    if off % 2 == 0:
                        iv = u0f[:, off:off + LEN]
                    else:
                        iv = u_odd[:, off - 1:off - 1 + LEN]
                    if t == 0:
                        nc.vector.tensor_scalar(acc[:], iv, w_ap, None, OP.mult)
                    else:
                        nc.vector.tensor_scalar(tmp[:], iv, w_ap, None, OP.mult)
                        nc.vector.tensor_tensor(acc[:], acc[:], tmp[:], OP.add)
                dwout0 = dw_pool.tile([128, CW], BF16, tag="dwout0")
                nc.vector.tensor_copy(
                    dwout0[:].rearrange("p (a b) -> p a b", a=hc),
                    acc[:].rearrange("p (a b) -> p a b", a=hc)[:, :, 2:2 + W])

                # -- depthwise tile1 on PE: 9 diagonal matmuls into PSUM --
                dwout1 = dw_pool.tile([128, CW], BF16, tag="dwout1")
                for pp in range(NT // 8):  # pairs of 512-chunks
                    psa = dg_ps.tile([128, 2, 256], F32, tag="dg_ps")
                    psb = dg_ps.tile([128, 2, 256], F32, tag="dg_ps")
                    for t, (dy, dx) in enumerate(taps):
                        st, sp_ = (t == 0), (t == 8)
                        for u, po in ((0, psa), (1, psb)):
                            j = 2 * pp + u
                            rv = u1[:, 2 + 2 * j + dy:4 + 2 * j + dy,
                                    2 + dx:2 + W + dx]
                            nc.tensor.matmul(po[:], dwdiag[:, t, :], rv,
                                             start=st, stop=sp_)
                    nc.scalar.copy(
                        dwout1[:, ts(2 * pp, 512)].rearrange(
                            "p (a b) -> p a b", a=2), psa[:])
                    nc.scalar.copy(
                        dwout1[:, ts(2 * pp + 1, 512)].rearrange(
                            "p (a b) -> p a b", a=2), psb[:])

                # -- depthwise packed tail on PE every 4th chunk --
                if g == 3:
                    tailout = to_pool.tile([128, CW], BF16, tag="tailout")
                    for pp in range(NT // 8):
                        psa = dg_ps.tile([128, 2, 256], F32, tag="dg_ps")
                        psb = dg_ps.tile([128, 2, 256], F32, tag="dg_ps")
                        for t, (dy, dx) in enumerate(taps):
                            st, sp_ = (t == 0), (t == 8)
                            for u, po in ((0, psa), (1, psb)):
                                j = 2 * pp + u
                                rv = tail_u[:, 2 + 2 * j + dy:4 + 2 * j + dy,
                                            2 + dx:2 + W + dx]
                                nc.tensor.matmul(po[:], dwdiag[:, 9 + t, :],
                                                 rv, start=st, stop=sp_)
                        nc.scalar.copy(
                            tailout[:, ts(2 * pp, 512)].rearrange(
                                "p (a b) -> p a b", a=2), psa[:])
                        nc.scalar.copy(
                            tailout[:, ts(2 * pp + 1, 512)].rearrange(
                                "p (a b) -> p a b", a=2), psb[:])
                    for gg in range(4):
                        cols = (ci - 3 + gg) * CW
                        nc.sync.dma_start(v_d[64:96, cols:cols + CW],
                                          tailout[32 * gg:32 * gg + 32, :])

                # -- v rows out --
                nc.sync.dma_start(v_d[0:32, ci * CW:(ci + 1) * CW],
                                  dwout0[96:128, :])
                nc.sync.dma_start(v_d[32:64, ci * CW:(ci + 1) * CW],
                                  dwout1[96:128, :])

                # -- q branch: W-row l2norm (tmp reused as Square scratch) --
                sqv = tmp[0:96, 0:CW]
                nc.scalar.activation(sqv, dwout0[0:96, :], AF.Square)
                rqc = rq_pool.tile([96, hc], F32, tag="rqc")
                nc.vector.tensor_reduce(
                    rqc[:], sqv.rearrange("p (a b) -> p a b", a=hc),
                    AX.X, OP.add)
                rnorm = rq_pool.tile([96, hc], F32, tag="rnorm")
                nc.scalar.sqrt(rnorm[:], rqc[:])
                rq = rq_pool.tile([96, hc], F32, tag="rq")
                nc.vector.reciprocal(rq[:], rnorm[:])

                # -- sp_mean for this chunk: im2col + matmul --
                imc = im_pool.tile([27, hc, W], BF16, tag="imc")
                for t, (dy, dx) in enumerate(taps):
                    nc.sync.dma_start(
                        imc[3 * t:3 * t + 3, :, :],
                        sp_pad[0:3, 1 + r0 + dy:1 + r0 + hc + dy,
                               2 + dx:2 + W + dx])
                spm = sm_pool.tile([96, CW], BF16, tag="spm")
                jj = 0
                while jj < CW // 512:
                    w2 = min(2, CW // 512 - jj)
                    ps = qkv_ps.tile([128, 4, 256], F32, tag="qkv_ps")
                    for u in range(w2):
                        rr = (jj + u) * 2
                        nc.tensor.matmul(ps[0:96, 2 * u:2 * u + 2, :],
                                         projw[:], imc[:, rr:rr + 2, :],
                                         start=True, stop=True)
                    nc.vector.tensor_copy(
                        spm[:, jj * 512:(jj + w2) * 512].rearrange(
                            "p (a b) -> p a b", a=2 * w2),
                        ps[0:96, 0:2 * w2, :])
                    jj += w2

                # -- qs_un = (qdw * rq_seg) * sp_mean, in place over dwout0 --
                for s in range(hc):
                    nc.vector.tensor_scalar(dwout0[0:96, ts(s, W)],
                                            dwout0[0:96, ts(s, W)],
                                            rq[:, s:s + 1], None, OP.mult)
                nc.vector.tensor_tensor(dwout0[0:96, :], dwout0[0:96, :],
                                        spm[:], OP.mult)

                # -- norm accumulators for hw-l2norm (post-gram absorb) --
                nc.scalar.activation(tmp[0:96, 0:CW], dwout0[0:96, :],
                                     AF.Square,
                                     accum_out=qn_cols[:, ci:ci + 1])
                nc.scalar.activation(tmp[0:96, 0:CW], dwout1[0:96, :],
                                     AF.Square,
                                     accum_out=kn_cols[:, ci:ci + 1])

                # -- xbar DMA transposes + gram accumulation --
                qsT = tr_pool.tile([128, NT, 96], BF16, tag="qsT")
                khT = tr_pool.tile([128, NT, 96], BF16, tag="khT")
                nc.sync.dma_start_transpose(qsT[:], dwout0[0:96, :])
                nc.sync.dma_start_transpose(khT[:], dwout1[0:96, :])
                for t in range(NT):
                    first = (ci == 0 and t == 0)
                    last = (ci == NCH - 1 and t == NT - 1)
                    nc.tensor.matmul(gram_ps[:], qsT[:, t, :], khT[:, t, :],
                                     start=first, stop=last)

        # ---- P2: attention block ----
        p2 = top.enter_context(tc.tile_pool(name="p2", bufs=1))
        ft_ps_pool = mid.enter_context(
            tc.tile_pool(name="ft_ps", bufs=1, space="PSUM"))

        qn = p2.tile([96, 1], F32, tag="qn")
        kn = p2.tile([96, 1], F32, tag="kn")
        nc.vector.tensor_reduce(qn[:], qn_cols[:], AX.X, OP.add)
        nc.vector.tensor_reduce(kn[:], kn_cols[:], AX.X, OP.add)
        nc.scalar.sqrt(qn[:], qn[:])
        nc.scalar.sqrt(kn[:], kn[:])
        rqf = p2.tile([96, 1], F32, tag="rqf")
        rkf = p2.tile([96, 1], F32, tag="rkf")
        nc.vector.reciprocal(rqf[:], qn[:])
        nc.vector.reciprocal(rkf[:], kn[:])
        nc.vector.tensor_tensor(rqf[:], rqf[:], par[0:96, 5:6], OP.mult)

        gram_sb = p2.tile([96, 96], F32, tag="gram_sb")
        nc.vector.tensor_copy(gram_sb[:], gram_ps[:])

        rkb = p2.tile([96, 32], F32, tag="rkb")
        nc.vector.tensor_scalar(rkb[:], rkf[:].broadcast_to((96, 32)), 1.0,
                                None, OP.mult)
        rkT = p2.tile([96, 32], F32, tag="rkT")
        nc.vector.transpose(rkT[:], rkb[:])

        attn = p2.tile([96, 32], F32, tag="attn")
        for h in range(3):
            nc.vector.tensor_copy(attn[32 * h:32 * h + 32, :],
                                  gram_sb[32 * h:32 * h + 32,
                                          32 * h:32 * h + 32])
        nc.vector.tensor_scalar(attn[:], attn[:], rqf[:], None, OP.mult)
        nc.vector.tensor_tensor(attn[:], attn[:], rkT[:], OP.mult)

        R = p2.tile([96, 32], F32, tag="R")
        gescr = p2.tile([96, 32], F32, tag="gescr")
        for d in range(32):
            nc.vector.tensor_scalar(gescr[:], attn[:], attn[:, d:d + 1], None,
                                    OP.is_ge)
            nc.vector.tensor_reduce(R[:, d:d + 1], gescr[:], AX.X, OP.add)

        A = p2.tile([96, 32], F32, tag="A")
        nc.vector.tensor_scalar(A[:], attn[:], 0.0, par[0:96, 4:5], OP.max,
                                OP.mult)
        mscr = p2.tile([96, 32], F32, tag="mscr")
        escr = p2.tile([96, 32], F32, tag="escr")
        mx = p2.tile([96, 1], F32, tag="mx")
        se = p2.tile([96, 1], F32, tag="se")
        coef = p2.tile([96, 1], F32, tag="coef")
        for i, kk in enumerate(KKS):
            nc.vector.tensor_scalar(mscr[:], R[:], float(kk), None, OP.is_le)
            nc.vector.tensor_scalar(mscr[:], mscr[:], 1.0 - SMALL, SMALL,
                                    OP.mult, OP.add)
            nc.vector.tensor_tensor(mscr[:], attn[:], mscr[:], OP.mult)
            nc.vector.tensor_reduce(mx[:], mscr[:], AX.X, OP.max)
            nc.vector.tensor_scalar(mscr[:], mscr[:], mx[:], None, OP.subtract)
            nc.scalar.activation(escr[:], mscr[:], AF.Exp, accum_out=se[:])
            nc.vector.reciprocal(se[:], se[:])
            nc.vector.tensor_tensor(coef[:], se[:], par[0:96, i:i + 1], OP.mult)
            nc.vector.scalar_tensor_tensor(A[:], escr[:], coef[:], A[:],
                                           OP.mult, OP.add)

        abd = p2.tile([96, 96], BF16, tag="abd")
        nc.vector.memset(abd[:], 0.0)
        for h in range(3):
            nc.vector.tensor_copy(abd[32 * h:32 * h + 32, 32 * h:32 * h + 32],
                                  A[32 * h:32 * h + 32, :])
        ft_ps = ft_ps_pool.tile([96, 192], F32)
        nc.tensor.matmul(ft_ps[:], abd[:], oww[:], start=True, stop=True)
        ftsb = p2.tile([96, 192], BF16, tag="ftsb")
        nc.scalar.copy(ftsb[:], ft_ps[:])

        mid.close()
        tc.strict_bb_all_engine_barrier()

        # ---- P3: partial = F @ v ----
        with ExitStack() as p3:
            v_pool = p3.enter_context(tc.tile_pool(name="v_pool", bufs=3))
            o_pool = p3.enter_context(tc.tile_pool(name="o_pool", bufs=2))
            o_ps = p3.enter_context(
                tc.tile_pool(name="o_ps", bufs=2, space="PSUM"))
            for j in range(HW // 1024):
                vsb = v_pool.tile([96, 1024], BF16, tag="vsb")
                nc.sync.dma_start(vsb[:], v_d[:, ts(j, 1024)])
                ps0 = o_ps.tile([128, 2, 512], F32, tag="ops0")
                ps1 = o_ps.tile([64, 2, 512], F32, tag="ops1")
                for u in range(2):
                    nc.tensor.matmul(ps0[:, u, :], ftsb[:, 0:128],
                                     vsb[:, ts(u, 512)], start=True, stop=True)
                    nc.tensor.matmul(ps1[:, u, :], ftsb[:, 128:192],
                                     vsb[:, ts(u, 512)], start=True, stop=True)
                ob0 = o_pool.tile([128, 1024], F32, tag="ob0")
                ob1 = o_pool.tile([64, 1024], F32, tag="ob1")
                nc.scalar.copy(ob0[:].rearrange("p (a b) -> p a b", a=2),
                               ps0[:])
                nc.vector.tensor_copy(
                    ob1[:].rearrange("p (a b) -> p a b", a=2), ps1[:])
                nc.sync.dma_start(out_d[0:128, ts(j, 1024)], ob0[:])
                nc.sync.dma_start(out_d[128:192, ts(j, 1024)], ob1[:])

    nc.compile()
    _NC_CACHE[key] = nc
    return nc


def core_inputs(i, x, superpixel_features, qkv_w, dw_w, proj_w, out_w,
                temperature, attn_scales, w_mix, Himg=HIMG):
    """Host-side slicing/packing of the full inputs for core i."""
    bf = ml_dtypes.bfloat16
    b, grp = i // 2, i % 2
    hs = 96 * grp
    HW = Himg * WIMG

    q_idx = np.arange(hs, hs + 96)
    k_idx = np.arange(DIM + hs, DIM + hs + 96)
    v_idx = np.arange(2 * DIM + hs, 2 * DIM + hs + 96)
    sel = np.concatenate([q_idx, v_idx[:32], k_idx, v_idx[32:64], v_idx[64:]])

    wq = np.asarray(qkv_w)[:, :, 0, 0]          # [576, 192]
    wqT = wq[sel].T.astype(bf)                  # [192, 288]

    w9 = np.asarray(dw_w)[:, 0].reshape(3 * DIM, 9)[sel]   # [288, 9]
    dww = w9[0:128].astype(np.float32)
    # diag matrices for tile1 (taps 0..8) and packed tail (taps 9..17)
    dwdiag = np.zeros((128, 18, 128), np.float32)
    tail = np.tile(w9[256:288], (4, 1))         # [128, 9]
    for t in range(9):
        np.fill_diagonal(dwdiag[:, t, :], w9[128:256, t])
        np.fill_diagonal(dwdiag[:, 9 + t, :], tail[:, t])
    dwdiag = dwdiag.astype(bf)

    pw = np.asarray(proj_w)[hs:hs + 96]          # [96, 3, 3, 3] (oc, ic, ky, kx)
    projw = np.transpose(pw, (2, 3, 1, 0)).reshape(27, 96).astype(bf)

    oww = np.asarray(out_w)[:, hs:hs + 96, 0, 0].T.astype(bf)   # [96, 192]

    wmx = np.exp(np.asarray(w_mix) - np.max(w_mix))
    wmx = (wmx / wmx.sum()).astype(np.float64)
    S = np.asarray(attn_scales, np.float64)
    par = np.zeros((128, 8), np.float32)
    for ii in range(4):
        par[:, ii] = wmx[1] * S[ii]
    par[:, 4] = wmx[0] * S.sum()
    temps = np.asarray(temperature).reshape(HEADS)[3 * grp:3 * grp + 3]
    par[0:96, 5] = np.repeat(temps, 32)

    return {
        "x": np.asarray(x)[b].reshape(DIM, HW).astype(bf),
        "wq0": wqT[:128].copy(),
        "wq1": wqT[128:].copy(),
        "dww": dww,
        "dwdiag": dwdiag,
        "projw": projw,
        "oww": oww,
        "sp": np.asarray(superpixel_features)[0, :, :Himg, :].astype(
            np.float32).copy(),
        "par": par,
    }


def kernel(x, superpixel_features, qkv_w, dw_w, proj_w, out_w, temperature,
           attn_scales, w_mix):
    nc = build_nc(HIMG, 16)
    in_maps = [
        core_inputs(i, x, superpixel_features, qkv_w, dw_w, proj_w, out_w,
                    temperature, attn_scales, w_mix)
        for i in range(N_CORES)
    ]
    res = run_bass_kernel_spmd(nc, in_maps, list(range(N_CORES)))
    out = np.empty((B, DIM, HIMG, WIMG), np.float32)
    for b in range(B):
        part = res.results[2 * b]["out"] + res.results[2 * b + 1]["out"]
        out[b] = part.reshape(DIM, HIMG, WIMG)
    return out
